# revision 12
# baseline (speedup 1.0000x reference)
"""Trainium2 Bass kernel for nn_Encoder (pre-norm transformer block, LN over
sequence axis) distributed over 8 NeuronCores.

v2 design (Megatron-TP, replicated x):
  - x replicated to every core in bf16 (plus own fp32 channel slice for the
    residual/stats); each core transposes the FULL x^T and applies LN1 with
    stats shared via a tiny [128,4] AllGather -> full h^T local, no big
    activation AllGather.
  - attention head-sharded (2 heads x 2 batches per core), scores computed
    transposed (S^T = k q^T), softmax denom via ones-column in V; exp runs on
    the Scalar engine in [128,1024] tiles (the attention-phase bottleneck);
    denominator reciprocal via PE broadcast + reciprocal_approx_fast on 64
    partitions.
  - Wo Megatron row-parallel: local partial y^T over all C from own heads,
    bf16 ReduceScatter(add) -> own channel slice; LN2 local.
  - FFN Megatron: W1 column-shard / W2 row-shard (1MB weights each, fully
    resident), AllGather(h2^T) in, bf16 ReduceScatter(z partials) out.
  - independent PE work (batch-1 transposes/QKV, Wo(b0)) interleaved into
    attention's exp-bound bubbles to keep the PE p-state high.
  - output channel-sharded [128, 4096]; host assembles + transposes.
"""

import numpy as np
import ml_dtypes
from contextlib import ExitStack

from concourse import bacc, bass_utils
import concourse.bass as bass
import concourse.tile as tile
import concourse.mybir as mybir
from concourse.masks import make_identity

FP32 = mybir.dt.float32
BF16 = mybir.dt.bfloat16
AF = mybir.ActivationFunctionType
ALU = mybir.AluOpType
AX = mybir.AxisListType

B, T, C, H, HS = 2, 2048, 1024, 16, 64
NCORE, P = 8, 128
TN = B * T            # 4096 flat tokens
F = 4 * C             # 4096
FL = F // NCORE       # 512 own FFN dims
MT = FL // P          # 4  own-f m-tiles
CM = C // P           # 8  chan m-tiles
KK = C // P           # 8  k-tiles over C
EPS = 1e-5
RG = [list(range(NCORE))]

_cache = {}


def _ln_stats(nc, pool, xsrc, g_sb, be_sb, A_out, B_out, n):
    """LN coefficients over the free axis of xsrc [P, n] into A_out/B_out
    ([P,1] APs): h = x*A + B. Unbiased var, eps outside sqrt."""
    s1 = pool.tile([P, 1], FP32, tag="s1")
    s2 = pool.tile([P, 1], FP32, tag="s2")
    nc.vector.reduce_sum(s1[:], xsrc, axis=AX.X)
    s2a = pool.tile([P, 1], FP32, tag="s2a")
    for ch in range(2):
        scr = pool.tile([P, n // 2], FP32, tag="scr")
        half = xsrc.rearrange("p (c n) -> p c n", c=2)[:, ch, :]
        nc.vector.scalar_tensor_tensor(
            out=scr[:], in0=half, scalar=1.0, in1=half,
            op0=ALU.mult, op1=ALU.mult,
            accum_out=(s2a[:] if ch == 0 else s2[:]))
    nc.vector.tensor_add(s2[:], s2[:], s2a[:])
    mean = pool.tile([P, 1], FP32, tag="mean")
    nc.vector.tensor_scalar_mul(mean[:], s1[:], 1.0 / n)
    ss = pool.tile([P, 1], FP32, tag="ss")
    nc.vector.tensor_mul(ss[:], s1[:], s1[:])
    var = pool.tile([P, 1], FP32, tag="var")
    nc.vector.scalar_tensor_tensor(
        out=var[:], in0=ss[:], scalar=-1.0 / n, in1=s2[:],
        op0=ALU.mult, op1=ALU.add)
    nc.vector.tensor_scalar_mul(var[:], var[:], 1.0 / (n - 1))
    den = pool.tile([P, 1], FP32, tag="den")
    nc.scalar.sqrt(den[:], var[:])
    nc.vector.tensor_scalar_add(den[:], den[:], EPS)
    rden = pool.tile([P, 1], FP32, tag="rden")
    nc.vector.reciprocal(rden[:], den[:])
    nc.vector.tensor_mul(A_out, g_sb, rden[:])
    mA = pool.tile([P, 1], FP32, tag="mA")
    nc.vector.tensor_scalar_mul(mA[:], mean[:], A_out)
    nc.vector.tensor_sub(B_out, be_sb, mA[:])


def build():
    nc = bacc.Bacc("TRN2", target_bir_lowering=False, debug=False,
                   num_devices=NCORE)

    def EIN(name, shape, dtype):
        return nc.dram_tensor(name, shape, dtype, kind="ExternalInput")

    x_bf = EIN("x_bf", [TN, C], BF16)      # full x, replicated
    x_c = EIN("x_c", [TN, P], FP32)        # own channel slice
    wq = EIN("wq", [P, KK, P], BF16)       # own 2 heads' Wq, kk-tiled
    wk = EIN("wk", [P, KK, P], BF16)
    wv = EIN("wv", [P, KK, P], BF16)
    wor = EIN("wor", [P, CM, P], BF16)     # Wo[own 128 rows,:] -> [p, m, mc]
    w1c = EIN("w1c", [P, KK, FL], BF16)    # W1[:, own cols] kk-tiled
    w2c = EIN("w2c", [P, MT, C], BF16)     # W2[own rows, :] q-tiled
    bqc = EIN("bqc", [P, 1], FP32)
    bkc = EIN("bkc", [P, 1], FP32)
    boc = EIN("boc", [P, 1], FP32)         # bo_eff own chans (post-reduce)
    b1c = EIN("b1c", [P, MT], FP32)
    b2c = EIN("b2c", [P, 1], FP32)
    g1 = EIN("g1", [P, 1], FP32)
    be1 = EIN("be1", [P, 1], FP32)
    g2 = EIN("g2", [P, 1], FP32)
    be2 = EIN("be2", [P, 1], FP32)
    out = nc.dram_tensor("out", [P, TN], FP32, kind="ExternalOutput")

    with tile.TileContext(nc) as tc, ExitStack() as ctx:
        const = ctx.enter_context(tc.tile_pool(name="const", bufs=1))
        dram = ctx.enter_context(tc.tile_pool(name="dram", bufs=1, space="DRAM"))
        persist = ctx.enter_context(tc.tile_pool(name="acts", bufs=1))
        stats = ctx.enter_context(tc.tile_pool(name="stats", bufs=2))
        # PSUM: wA 2x[128,1024]f32 (4 banks) + wS [64,512] (1) + att [65,1024]
        # (2) + tpp [128,1024]bf16 (1) = 8 banks
        wA = ctx.enter_context(tc.tile_pool(name="wA", bufs=2, space="PSUM"))
        wS = ctx.enter_context(tc.tile_pool(name="wS", bufs=1, space="PSUM"))

        idf = const.tile([P, P], FP32)
        make_identity(nc, idf)
        idb = const.tile([P, P], BF16)
        make_identity(nc, idb)
        ones_b = const.tile([1, P], BF16)
        nc.vector.memset(ones_b[:], 1.0)

        def ldconst(pool, t, shape, dt=FP32, eng=None):
            s = pool.tile(shape, dt, name=t.name + "_sb")
            (eng or nc.gpsimd).dma_start(s[:], t.ap())
            return s

        # g1/be1 needed first (stats); heavier weights loaded after x below
        g1_sb = ldconst(const, g1, [P, 1])
        be1_sb = ldconst(const, be1, [P, 1])

        # persistent activations
        xT = persist.tile([P, B, T], FP32)        # own chans, transposed
        yT = persist.tile([P, B, T], FP32)
        h2T = persist.tile([P, B, T], BF16)
        st_sb = persist.tile([P, 2 * B], FP32)    # own A/B for b0,b1
        ag_sb = persist.tile([P, KK, 2 * B], FP32)  # gathered stats

        # DRAM comm tiles
        st_in = dram.tile([P, 2 * B], FP32, name="st_in")
        st_out = dram.tile([C, 2 * B], FP32, addr_space="Shared", name="st_out")
        rsy_in = [dram.tile([NCORE, P, T], BF16, name=f"rsy_in{b}")
                  for b in range(B)]
        rsy_out = [dram.tile([P, T], BF16, name=f"rsy_out{b}")
                   for b in range(B)]
        h2_in = [dram.tile([P, T], BF16, name=f"h2_in{b}") for b in range(B)]
        h2_out = [dram.tile([C, T], BF16, addr_space="Shared",
                            name=f"h2_out{b}") for b in range(B)]
        rsz_in = [dram.tile([NCORE, P, T], BF16, name=f"rsz_in{b}")
                  for b in range(B)]
        rsz_out = [dram.tile([P, T], BF16, name=f"rsz_out{b}")
                   for b in range(B)]

        # PSUM drains alternate vector / scalar-copy (gpsimd cannot
        # touch PSUM on hardware)
        _rr = [0]

        def zdrain():
            _rr[0] ^= 1
            return nc.vector.tensor_copy if _rr[0] else nc.scalar.copy

        # ---- LN2 units (y residual + stats + apply + AG trigger) ----
        def ln2_units(b):
            def u1():
                ys = fin.tile([P, T], BF16, tag="ys", name=f"ys{b}")
                nc.sync.dma_start(ys[:], rsy_out[b][:])
                nc.vector.scalar_tensor_tensor(
                    out=yT[:, b, :], in0=ys[:], scalar=bo_sb[:],
                    in1=xT[:, b, :], op0=ALU.add, op1=ALU.add)

            def u2():
                A2 = stats.tile([P, 1], FP32, tag="A2")
                B2 = stats.tile([P, 1], FP32, tag="B2")
                _ln_stats(nc, stats, yT[:, b, :], g2_sb[:], be2_sb[:],
                          A2[:], B2[:], T)
                nc.vector.tensor_scalar(
                    out=h2T[:, b, :], in0=yT[:, b, :],
                    scalar1=A2[:], scalar2=B2[:], op0=ALU.mult, op1=ALU.add)
                nc.sync.dma_start(h2_in[b][:], h2T[:, b, :])
                nc.gpsimd.collective_compute(
                    "AllGather", ALU.bypass, replica_groups=RG,
                    ins=[h2_in[b].opt()], outs=[h2_out[b].opt()])
                h2full[b] = h2fp.tile([P, KK, T], BF16, tag="h2f",
                                      name=f"h2full{b}")
                nc.sync.dma_start(
                    h2full[b][:],
                    h2_out[b].rearrange("(kk p) n -> p kk n", p=P))
            return [u1, u2]

        h2full = {}

        with tc.tile_pool(name="attq", bufs=2, space="PSUM") as attq, \
             tc.tile_pool(name="psb", bufs=4) as psb, \
             tc.tile_pool(name="qkvp", bufs=1) as qkvp, \
             tc.tile_pool(name="stg", bufs=4) as stg, \
             tc.tile_pool(name="small", bufs=3) as small:

            qT = qkvp.tile([P, B, T], BF16)
            kT = qkvp.tile([P, B, T], BF16)
            vaug = qkvp.tile([P, B, T // P, 130], BF16)
            attnT = qkvp.tile([P, B, T], BF16)

            hTt = {}

            def hT_of(b):
                if b not in hTt:
                    hTt[b] = hTp.tile([P, KK, T], BF16, tag="hT",
                                      name=f"hT{b}")
                return hTt[b]

            # ---- QKV units ----
            def qk_unit(b, w_sb, bias_sb, dst, j):
                def u():
                    hT = hT_of(b)
                    ps = wA.tile([P, 512], FP32, tag="wa", name="qkps")
                    for kk in range(KK):
                        nc.tensor.matmul(
                            ps[:], lhsT=w_sb[:, kk, :],
                            rhs=hT[:, kk, j * 512:(j + 1) * 512],
                            start=(kk == 0), stop=(kk == KK - 1))
                    nc.vector.tensor_scalar_add(
                        dst[:, b, j * 512:(j + 1) * 512], ps[:], bias_sb[:])
                return u

            def v_unit(b, tg):       # tg in 0..3, covers 4 tt
                def u():
                    hT = hT_of(b)
                    ps = wA.tile([P, 512], FP32, tag="wa", name="vps")
                    for q in range(4):
                        tt = tg * 4 + q
                        for kk in range(KK):
                            nc.tensor.matmul(
                                ps[:, q * P:(q + 1) * P],
                                lhsT=hT[:, kk, tt * P:(tt + 1) * P],
                                rhs=wv_sb[:, kk, :],
                                start=(kk == 0), stop=(kk == KK - 1))
                    dst = vaug[:, b, tg * 4:(tg + 1) * 4, :].rearrange(
                        "p tt (h x) -> p tt h x", h=2)[:, :, :, 0:64]
                    srcv = ps[:].rearrange("p (tt h x) -> p tt h x",
                                           tt=4, h=2)
                    nc.vector.tensor_copy(dst, srcv)
                return u

            def qkv_units(b):
                us = []
                for j in range(4):
                    us.append(qk_unit(b, wq_sb, bq_sb, qT, j))
                    us.append(qk_unit(b, wk_sb, bk_sb, kT, j))
                for tg in range(4):
                    us.append(v_unit(b, tg))
                return us

            def rs_y(b):
                nc.gpsimd.collective_compute(
                    "ReduceScatter", ALU.add, replica_groups=RG,
                    ins=[rsy_in[b].opt()], outs=[rsy_out[b].opt()])

            # ---- attention chunk (b, j): both heads then inline Wo ----
            def attention_chunk(b, j, fillers):
                jsl = slice(j * 512, (j + 1) * 512)
                for hd in range(2):
                    h64 = slice(hd * 64, (hd + 1) * 64)
                    att = attq.tile([65, 512], FP32, tag="att",
                                    name=f"att{b}{j}{hd}")
                    for k in range(T // P):
                        if fillers and (k % 2 == 0):
                            fillers.pop(0)()
                        sc = wA.tile([P, 512], FP32, tag="wa", name="sc")
                        nc.tensor.matmul(
                            sc[:], lhsT=kT[h64, b, k * P:(k + 1) * P],
                            rhs=qT[h64, b, jsl], start=True, stop=True)
                        p = psb.tile([P, 512], BF16, tag="p")
                        nc.scalar.activation(p[:], sc[:], AF.Exp,
                                             scale=float(HS) ** -0.5)
                        nc.tensor.matmul(
                            att[:], lhsT=vaug[:, b, k, hd * 65:(hd + 1) * 65],
                            rhs=p[:], start=(k == 0), stop=(k == T // P - 1))
                    # normalize: denom bcast via PE + approx reciprocal
                    den = small.tile([1, 512], BF16, tag="den")
                    nc.vector.tensor_copy(den[:], att[64:65, :])
                    rdp = wA.tile([P, 512], FP32, tag="wa", name="rdp")
                    nc.tensor.matmul(rdp[0:64, :], lhsT=ones_b[:, 0:64],
                                     rhs=den[:], start=True, stop=True)
                    rd = small.tile([64, 512], FP32, tag="rd")
                    nc.vector.reciprocal_approx_fast(rd[:], rdp[0:64, :])
                    nc.vector.tensor_mul(attnT[h64, b, jsl], att[0:64, :],
                                         rd[:])
                # inline Wo for this token chunk (local partial, all chans)
                for m in range(CM):
                    ps = wA.tile([P, 512], FP32, tag="wa", name="wops")
                    nc.tensor.matmul(ps[:], lhsT=wor_sb[:, m, :],
                                     rhs=attnT[:, b, jsl],
                                     start=True, stop=True)
                    st = stg.tile([P, 512], BF16, tag="wos")
                    zdrain()(st[:], ps[:])
                    nc.sync.dma_start(rsy_in[b][m][:, jsl], st[:])

            # ---- FFN units (Megatron column/row parallel) ----
            uTt = {}

            def w1_unit(b, j, m):
                def u():
                    if m == 0:
                        uTt[(b, j)] = upool.tile([P, MT, 512], BF16, tag="u",
                                                 name=f"uT{b}{j}")
                    ps = wA.tile([P, 512], FP32, tag="wa", name="w1ps")
                    for kk in range(KK):
                        nc.tensor.matmul(
                            ps[:], lhsT=w1_sb[0][:, kk, m * P:(m + 1) * P],
                            rhs=h2full[b][:, kk, j * 512:(j + 1) * 512],
                            start=(kk == 0), stop=(kk == KK - 1))
                    nc.scalar.activation(uTt[(b, j)][:, m, :], ps[:], AF.Relu,
                                         bias=b1_sb[:, m:m + 1], scale=1.0)
                return u

            def w2_unit(b, j, mc):
                def u():
                    ps = wA.tile([P, 512], FP32, tag="wa", name="w2ps")
                    for q in range(MT):
                        nc.tensor.matmul(
                            ps[:], lhsT=w2_sb[0][:, q, mc * P:(mc + 1) * P],
                            rhs=uTt[(b, j)][:, q, :],
                            start=(q == 0), stop=(q == MT - 1))
                    st = stg.tile([P, 512], BF16, tag="zs")
                    zdrain()(st[:], ps[:])
                    nc.sync.dma_start(rsz_in[b][mc][:, j * 512:(j + 1) * 512],
                                      st[:])
                return u

            def ffn_units(b, j):
                return ([w1_unit(b, j, m) for m in range(MT)]
                        + [w2_unit(b, j, mc) for mc in range(CM)])

            def rs_z(b):
                nc.gpsimd.collective_compute(
                    "ReduceScatter", ALU.add, replica_groups=RG,
                    ins=[rsz_in[b].opt()], outs=[rsz_out[b].opt()])

            def final(b):
                zs = fin.tile([P, T], BF16, tag="zf", name=f"zf{b}")
                nc.sync.dma_start(zs[:], rsz_out[b][:])
                o = fin.tile([P, T], FP32, tag="o", name=f"o{b}")
                nc.vector.scalar_tensor_tensor(
                    out=o[:], in0=zs[:], scalar=b2_sb[:],
                    in1=yT[:, b, :], op0=ALU.add, op1=ALU.add)
                nc.sync.dma_start(out.ap()[:, b * T:(b + 1) * T], o[:])

            # =========== emission ===========
            with tc.tile_pool(name="xst", bufs=2) as xstp, \
                 tc.tile_pool(name="tpp", bufs=2, space="PSUM") as tpp, \
                 tc.tile_pool(name="hT", bufs=1) as hTp:

                # phase A: own-slice chunks first (critical path: stats AG)
                with tc.tile_pool(name="xc", bufs=1) as xcp:
                    xc_sb = {}
                    for b in range(B):
                        xc_sb[b] = xcp.tile([P, T // P, P], FP32, tag="xc",
                                            name=f"xc{b}")
                        nc.sync.dma_start(
                            xc_sb[b][:],
                            x_c.ap()[b * T:(b + 1) * T, :]
                            .rearrange("(tt p) c -> p tt c", p=P))
                    # full-x stages + weights (gpsimd queue, after xc)
                    xst = {}
                    for b in range(B):
                        for hf in range(2):
                            s = xstp.tile([P, 8, C], BF16, tag="xst",
                                          name=f"xst{b}{hf}")
                            nc.gpsimd.dma_start(
                                s[:],
                                x_bf.ap()[b * T + hf * 1024:
                                          b * T + (hf + 1) * 1024, :]
                                .rearrange("(tt p) c -> p tt c", p=P))
                            xst[(b, hf)] = s
                    wq_sb = ldconst(const, wq, [P, KK, P], BF16)
                    wk_sb = ldconst(const, wk, [P, KK, P], BF16)
                    wv_sb = ldconst(const, wv, [P, KK, P], BF16)
                    wor_sb = ldconst(const, wor, [P, CM, P], BF16)
                    bq_sb = ldconst(const, bqc, [P, 1])
                    bk_sb = ldconst(const, bkc, [P, 1])
                    bo_sb = ldconst(const, boc, [P, 1])
                    b1_sb = ldconst(const, b1c, [P, MT])
                    b2_sb = ldconst(const, b2c, [P, 1])
                    g2_sb = ldconst(const, g2, [P, 1])
                    be2_sb = ldconst(const, be2, [P, 1])
                    w1_sb = [ldconst(const, w1c, [P, KK, FL], BF16,
                                     eng=nc.scalar)]
                    w2_sb = [ldconst(const, w2c, [P, MT, C], BF16,
                                     eng=nc.scalar)]
                    nc.vector.memset(vaug[:, :, :, 64], 1.0)
                    nc.vector.memset(vaug[:, :, :, 129], 1.0)

                    for b in range(B):
                        for g in range(4):   # 4 transposes per wA tile
                            tp = wA.tile([P, 512], FP32, tag="wa",
                                         name=f"xtp{b}{g}")
                            for q in range(4):
                                tt = g * 4 + q
                                nc.tensor.transpose(
                                    tp[:, q * P:(q + 1) * P],
                                    xc_sb[b][:, tt, :], idf[:])
                            nc.vector.tensor_copy(
                                xT[:, b, g * 512:(g + 1) * 512], tp[:])
                        _ln_stats(nc, stats, xT[:, b, :], g1_sb[:], be1_sb[:],
                                  st_sb[:, 2 * b:2 * b + 1],
                                  st_sb[:, 2 * b + 1:2 * b + 2], T)
                    nc.sync.dma_start(st_in[:], st_sb[:])
                    nc.gpsimd.collective_compute(
                        "AllGather", ALU.bypass, replica_groups=RG,
                        ins=[st_in.opt()], outs=[st_out.opt()])
                    nc.sync.dma_start(
                        ag_sb[:], st_out.rearrange("(kk p) n -> p kk n", p=P))

                # transpose raw x^T into hT (no stats dep), apply in-place
                def trans_unit(b, kk, half):
                    def u():
                        hT = hT_of(b)
                        tp = tpp.tile([P, 1024], BF16, tag="tp")
                        for q in range(8):
                            tt = half * 8 + q
                            nc.tensor.transpose(
                                tp[:, q * P:(q + 1) * P],
                                xst[(b, tt // 8)][:, tt % 8,
                                                  kk * P:(kk + 1) * P],
                                idb[:])
                        nc.vector.tensor_copy(
                            hT[:, kk, half * 1024:(half + 1) * 1024], tp[:])
                    return u

                def apply_unit(b, kk):
                    def u():
                        hT = hT_of(b)
                        nc.vector.tensor_scalar(
                            out=hT[:, kk, :], in0=hT[:, kk, :],
                            scalar1=ag_sb[:, kk, 2 * b:2 * b + 1],
                            scalar2=ag_sb[:, kk, 2 * b + 1:2 * b + 2],
                            op0=ALU.mult, op1=ALU.add)
                    return u

                # b0: raw transposes, then stats-gated in-place applies
                for kk in range(KK):
                    for half in range(2):
                        trans_unit(0, kk, half)()
                for kk in range(KK):
                    apply_unit(0, kk)()
                for u in qkv_units(0):
                    u()

                # attention(b0): Wo inline per chunk; b1 prep as fillers
                fill_b0 = []
                for kk in range(KK):
                    for half in range(2):
                        fill_b0.append(trans_unit(1, kk, half))
                    fill_b0.append(apply_unit(1, kk))
                fill_b0.extend(qkv_units(1))
                for j in range(4):
                    attention_chunk(0, j, fill_b0)
                while fill_b0:
                    fill_b0.pop(0)()
                rs_y(0)

            with tc.tile_pool(name="h2f", bufs=1) as h2fp, \
                 tc.tile_pool(name="upool", bufs=2) as upool, \
                 tc.tile_pool(name="fin", bufs=2) as fin:
                # attention(b1): LN2(b0) after chunk 0; FFN(b0,j0) fillers in
                # the last chunk (h2full(b0) lands mid-b1)
                l2u0 = ln2_units(0)
                fill_b1 = []
                attention_chunk(1, 0, fill_b1)
                l2u0[0]()
                l2u0[1]()
                attention_chunk(1, 1, fill_b1)
                attention_chunk(1, 2, fill_b1)
                fill_last = ffn_units(0, 0)
                attention_chunk(1, 3, fill_last)
                while fill_last:
                    fill_last.pop(0)()
                rs_y(1)

                # FFN(b0) rest; LN2(b1) once rsy_out(b1) landed; FFN(b1)
                for u in ffn_units(0, 1):
                    u()
                for u in ffn_units(0, 2):
                    u()
                l2u1 = ln2_units(1)
                l2u1[0]()
                for u in ffn_units(0, 3):
                    u()
                rs_z(0)
                l2u1[1]()
                for j in range(4):
                    for u in ffn_units(1, j):
                        u()
                rs_z(1)
                final(0)
                final(1)

    nc.compile()
    return nc


def prep_inputs(x, Wq, bq, Wk, bk, Wv, bv, Wo, bo, W1, b1, W2, b2,
                gamma1, beta1, gamma2, beta2):
    bf = ml_dtypes.bfloat16
    xf = np.asarray(x, np.float32).reshape(TN, C)
    x_bf_full = np.ascontiguousarray(xf.astype(bf))
    # softmax rows sum to 1, so the v bias is equivalent to adding
    # concat_h(bv) @ Wo to the attention-projection bias
    bo_eff = (np.asarray(bo, np.float64)
              + np.asarray(bv, np.float64).reshape(C) @ np.asarray(Wo, np.float64)
              ).astype(np.float32)
    Wo = np.asarray(Wo, np.float32)
    W1 = np.asarray(W1, np.float32)
    W2 = np.asarray(W2, np.float32)
    in_maps = []
    for i in range(NCORE):
        ci = slice(P * i, P * (i + 1))
        fi = slice(FL * i, FL * (i + 1))
        hA, hB = 2 * i, 2 * i + 1

        def tile_km(wcat):  # [C, 128] -> [p, kk, m]
            return np.ascontiguousarray(
                wcat.reshape(KK, P, P).transpose(1, 0, 2)).astype(bf)

        wq_cat = np.concatenate([Wq[hA], Wq[hB]], axis=1)
        wk_cat = np.concatenate([Wk[hA], Wk[hB]], axis=1)
        wv_cat = np.concatenate([Wv[hA], Wv[hB]], axis=1)
        in_maps.append({
            "x_bf": x_bf_full,
            "x_c": np.ascontiguousarray(xf[:, ci]),
            "wq": tile_km(wq_cat),
            "wk": tile_km(wk_cat),
            "wv": tile_km(wv_cat),
            "wor": np.ascontiguousarray(
                Wo[ci, :].reshape(P, CM, P)).astype(bf),
            "w1c": np.ascontiguousarray(
                W1[:, fi].reshape(KK, P, FL).transpose(1, 0, 2)).astype(bf),
            "w2c": np.ascontiguousarray(
                W2[fi, :].reshape(MT, P, C).transpose(1, 0, 2)).astype(bf),
            "bqc": np.concatenate([bq[hA], bq[hB]])[:, None].astype(np.float32),
            "bkc": np.concatenate([bk[hA], bk[hB]])[:, None].astype(np.float32),
            "boc": bo_eff[ci][:, None].astype(np.float32),
            "b1c": np.ascontiguousarray(
                np.asarray(b1)[fi].reshape(MT, P).T).astype(np.float32),
            "b2c": np.asarray(b2)[ci][:, None].astype(np.float32),
            "g1": np.asarray(gamma1)[ci][:, None].astype(np.float32),
            "be1": np.asarray(beta1)[ci][:, None].astype(np.float32),
            "g2": np.asarray(gamma2)[ci][:, None].astype(np.float32),
            "be2": np.asarray(beta2)[ci][:, None].astype(np.float32),
        })
    return in_maps


def kernel(**inputs):
    inputs = {k: np.asarray(v) for k, v in inputs.items()}
    if "nc" not in _cache:
        _cache["nc"] = build()
    nc = _cache["nc"]
    in_maps = prep_inputs(**inputs)
    res = bass_utils.run_bass_kernel_spmd(nc, in_maps, core_ids=list(range(NCORE)))
    full = np.concatenate([res.results[i]["out"] for i in range(NCORE)], axis=0)
    return np.ascontiguousarray(full.T).reshape(B, T, C).astype(np.float32)


# revision 15
# speedup vs baseline: 1.0683x; 1.0683x over previous
"""Trainium2 Bass kernel for nn_Encoder (pre-norm transformer block, LN over
sequence axis) distributed over 8 NeuronCores.

v2 design (Megatron-TP, replicated x):
  - x replicated to every core in bf16 (plus own fp32 channel slice for the
    residual/stats); each core transposes the FULL x^T and applies LN1 with
    stats shared via a tiny [128,4] AllGather -> full h^T local, no big
    activation AllGather.
  - attention head-sharded (2 heads x 2 batches per core), scores computed
    transposed (S^T = k q^T), softmax denom via ones-column in V; exp runs on
    the Scalar engine in [128,1024] tiles (the attention-phase bottleneck);
    denominator reciprocal via PE broadcast + reciprocal_approx_fast on 64
    partitions.
  - Wo Megatron row-parallel: local partial y^T over all C from own heads,
    bf16 ReduceScatter(add) -> own channel slice; LN2 local.
  - FFN Megatron: W1 column-shard / W2 row-shard (1MB weights each, fully
    resident), AllGather(h2^T) in, bf16 ReduceScatter(z partials) out.
  - independent PE work (batch-1 transposes/QKV, Wo(b0)) interleaved into
    attention's exp-bound bubbles to keep the PE p-state high.
  - output channel-sharded [128, 4096]; host assembles + transposes.
"""

import numpy as np
import ml_dtypes
from contextlib import ExitStack

from concourse import bacc, bass_utils
import concourse.bass as bass
import concourse.tile as tile
import concourse.mybir as mybir
from concourse.masks import make_identity

FP32 = mybir.dt.float32
BF16 = mybir.dt.bfloat16
AF = mybir.ActivationFunctionType
ALU = mybir.AluOpType
AX = mybir.AxisListType

B, T, C, H, HS = 2, 2048, 1024, 16, 64
NCORE, P = 8, 128
TN = B * T            # 4096 flat tokens
F = 4 * C             # 4096
FL = F // NCORE       # 512 own FFN dims
MT = FL // P          # 4  own-f m-tiles
CM = C // P           # 8  chan m-tiles
KK = C // P           # 8  k-tiles over C
EPS = 1e-5
RG = [list(range(NCORE))]

_cache = {}


def _ln_stats(nc, pool, xsrc, g_sb, be_sb, A_out, B_out, n):
    """LN coefficients over the free axis of xsrc [P, n] into A_out/B_out
    ([P,1] APs): h = x*A + B. Unbiased var, eps outside sqrt."""
    s1 = pool.tile([P, 1], FP32, tag="s1")
    s2 = pool.tile([P, 1], FP32, tag="s2")
    nc.vector.reduce_sum(s1[:], xsrc, axis=AX.X)
    s2a = pool.tile([P, 1], FP32, tag="s2a")
    for ch in range(2):
        scr = pool.tile([P, n // 2], FP32, tag="scr")
        half = xsrc.rearrange("p (c n) -> p c n", c=2)[:, ch, :]
        nc.vector.scalar_tensor_tensor(
            out=scr[:], in0=half, scalar=1.0, in1=half,
            op0=ALU.mult, op1=ALU.mult,
            accum_out=(s2a[:] if ch == 0 else s2[:]))
    nc.vector.tensor_add(s2[:], s2[:], s2a[:])
    mean = pool.tile([P, 1], FP32, tag="mean")
    nc.vector.tensor_scalar_mul(mean[:], s1[:], 1.0 / n)
    ss = pool.tile([P, 1], FP32, tag="ss")
    nc.vector.tensor_mul(ss[:], s1[:], s1[:])
    var = pool.tile([P, 1], FP32, tag="var")
    nc.vector.scalar_tensor_tensor(
        out=var[:], in0=ss[:], scalar=-1.0 / n, in1=s2[:],
        op0=ALU.mult, op1=ALU.add)
    nc.vector.tensor_scalar_mul(var[:], var[:], 1.0 / (n - 1))
    den = pool.tile([P, 1], FP32, tag="den")
    nc.scalar.sqrt(den[:], var[:])
    nc.vector.tensor_scalar_add(den[:], den[:], EPS)
    rden = pool.tile([P, 1], FP32, tag="rden")
    nc.vector.reciprocal(rden[:], den[:])
    nc.vector.tensor_mul(A_out, g_sb, rden[:])
    mA = pool.tile([P, 1], FP32, tag="mA")
    nc.vector.tensor_scalar_mul(mA[:], mean[:], A_out)
    nc.vector.tensor_sub(B_out, be_sb, mA[:])


def build():
    nc = bacc.Bacc("TRN2", target_bir_lowering=False, debug=False,
                   num_devices=NCORE)

    def EIN(name, shape, dtype):
        return nc.dram_tensor(name, shape, dtype, kind="ExternalInput")

    x_bf = EIN("x_bf", [TN, C], BF16)      # full x, replicated
    x_c = EIN("x_c", [TN, P], FP32)        # own channel slice
    wq = EIN("wq", [P, KK, P], BF16)       # own 2 heads' Wq, kk-tiled
    wk = EIN("wk", [P, KK, P], BF16)
    wv = EIN("wv", [P, KK, P], BF16)
    wor = EIN("wor", [P, CM, P], BF16)     # Wo[own 128 rows,:] -> [p, m, mc]
    w1c = EIN("w1c", [P, KK, FL], BF16)    # W1[:, own cols] kk-tiled
    w2c = EIN("w2c", [P, MT, C], BF16)     # W2[own rows, :] q-tiled
    bqc = EIN("bqc", [P, 1], FP32)
    bkc = EIN("bkc", [P, 1], FP32)
    boc = EIN("boc", [P, 1], FP32)         # bo_eff own chans (post-reduce)
    b1c = EIN("b1c", [P, MT], FP32)
    b2c = EIN("b2c", [P, 1], FP32)
    g1 = EIN("g1", [P, 1], FP32)
    be1 = EIN("be1", [P, 1], FP32)
    g2 = EIN("g2", [P, 1], FP32)
    be2 = EIN("be2", [P, 1], FP32)
    out = nc.dram_tensor("out", [P, TN], FP32, kind="ExternalOutput")

    with tile.TileContext(nc) as tc, ExitStack() as ctx:
        const = ctx.enter_context(tc.tile_pool(name="const", bufs=1))
        dram = ctx.enter_context(tc.tile_pool(name="dram", bufs=1, space="DRAM"))
        persist = ctx.enter_context(tc.tile_pool(name="acts", bufs=1))
        stats = ctx.enter_context(tc.tile_pool(name="stats", bufs=2))
        # PSUM: wA 2x[128,1024]f32 (4 banks) + wS [64,512] (1) + att [65,1024]
        # (2) + tpp [128,1024]bf16 (1) = 8 banks
        wA = ctx.enter_context(tc.tile_pool(name="wA", bufs=2, space="PSUM"))
        wS = ctx.enter_context(tc.tile_pool(name="wS", bufs=1, space="PSUM"))

        idf = const.tile([P, P], FP32)
        make_identity(nc, idf)
        idb = const.tile([P, P], BF16)
        make_identity(nc, idb)
        ones_b = const.tile([1, P], BF16)
        nc.vector.memset(ones_b[:], 1.0)

        def ldconst(pool, t, shape, dt=FP32, eng=None):
            s = pool.tile(shape, dt, name=t.name + "_sb")
            (eng or nc.gpsimd).dma_start(s[:], t.ap())
            return s

        # g1/be1 needed first (stats); heavier weights loaded after x below
        g1_sb = ldconst(const, g1, [P, 1])
        be1_sb = ldconst(const, be1, [P, 1])

        # persistent activations
        xT = persist.tile([P, B, T], FP32)        # own chans, transposed
        yT = persist.tile([P, B, T], FP32)
        h2T = persist.tile([P, B, T], BF16)
        st_sb = persist.tile([P, 2 * B], FP32)    # own A/B for b0,b1
        ag_sb = persist.tile([P, KK, 2 * B], FP32)  # gathered stats

        # DRAM comm tiles
        st_in = dram.tile([P, 2 * B], FP32, name="st_in")
        st_out = dram.tile([C, 2 * B], FP32, addr_space="Shared", name="st_out")
        rsy_in = [dram.tile([NCORE, P, T], BF16, name=f"rsy_in{b}")
                  for b in range(B)]
        rsy_out = [dram.tile([P, T], BF16, name=f"rsy_out{b}")
                   for b in range(B)]
        h2_in = [dram.tile([P, T], BF16, name=f"h2_in{b}") for b in range(B)]
        h2_out = [dram.tile([C, T], BF16, addr_space="Shared",
                            name=f"h2_out{b}") for b in range(B)]
        rsz_in = [dram.tile([NCORE, P, T], BF16, name=f"rsz_in{b}")
                  for b in range(B)]
        rsz_out = [dram.tile([P, T], BF16, name=f"rsz_out{b}")
                   for b in range(B)]

        # PSUM drains alternate vector / scalar-copy (gpsimd cannot
        # touch PSUM on hardware)
        _rr = [0]

        def zdrain():
            _rr[0] ^= 1
            return nc.vector.tensor_copy if _rr[0] else nc.scalar.copy

        # ---- LN2 units (y residual + stats + apply + AG trigger) ----
        def ln2_units(b):
            def u1():
                ys = fin.tile([P, T], BF16, tag="ys", name=f"ys{b}")
                nc.sync.dma_start(ys[:], rsy_out[b][:])
                nc.vector.scalar_tensor_tensor(
                    out=yT[:, b, :], in0=ys[:], scalar=bo_sb[:],
                    in1=xT[:, b, :], op0=ALU.add, op1=ALU.add)

            def u2():
                A2 = stats.tile([P, 1], FP32, tag="A2")
                B2 = stats.tile([P, 1], FP32, tag="B2")
                _ln_stats(nc, stats, yT[:, b, :], g2_sb[:], be2_sb[:],
                          A2[:], B2[:], T)
                nc.vector.tensor_scalar(
                    out=h2T[:, b, :], in0=yT[:, b, :],
                    scalar1=A2[:], scalar2=B2[:], op0=ALU.mult, op1=ALU.add)
                nc.sync.dma_start(h2_in[b][:], h2T[:, b, :])
                nc.gpsimd.collective_compute(
                    "AllGather", ALU.bypass, replica_groups=RG,
                    ins=[h2_in[b].opt()], outs=[h2_out[b].opt()])
                h2full[b] = h2fp.tile([P, KK, T], BF16, tag="h2f",
                                      name=f"h2full{b}")
                h2v = h2_out[b].rearrange("(kk p) n -> p kk n", p=P)
                for jj in range(4):
                    nc.sync.dma_start(
                        h2full[b][:, :, jj * 512:(jj + 1) * 512],
                        h2v[:, :, jj * 512:(jj + 1) * 512])
            return [u1, u2]

        h2full = {}

        with tc.tile_pool(name="attq", bufs=2, space="PSUM") as attq, \
             tc.tile_pool(name="psb", bufs=12) as psb, \
             tc.tile_pool(name="qkvp", bufs=1) as qkvp, \
             tc.tile_pool(name="stg", bufs=4) as stg, \
             tc.tile_pool(name="small", bufs=3) as small:

            qT = qkvp.tile([P, B, T], BF16)
            kT = qkvp.tile([P, B, T], BF16)
            vaug = qkvp.tile([P, B, T // P, 130], BF16)
            attnT = qkvp.tile([P, B, T], BF16)

            hTt = {}

            def hT_of(b):
                if b not in hTt:
                    hTt[b] = hTp.tile([P, KK, T], BF16, tag="hT",
                                      name=f"hT{b}")
                return hTt[b]

            # ---- QKV units ----
            def qk_unit(b, w_sb, bias_sb, dst, j):
                def u():
                    hT = hT_of(b)
                    ps = wA.tile([P, 512], FP32, tag="wa", name="qkps")
                    for kk in range(KK):
                        nc.tensor.matmul(
                            ps[:], lhsT=w_sb[:, kk, :],
                            rhs=hT[:, kk, j * 512:(j + 1) * 512],
                            start=(kk == 0), stop=(kk == KK - 1))
                    nc.vector.tensor_scalar_add(
                        dst[:, b, j * 512:(j + 1) * 512], ps[:], bias_sb[:])
                return u

            def v_unit(b, tg):       # tg in 0..3, covers 4 tt
                def u():
                    hT = hT_of(b)
                    ps = wA.tile([P, 512], FP32, tag="wa", name="vps")
                    for q in range(4):
                        tt = tg * 4 + q
                        for kk in range(KK):
                            nc.tensor.matmul(
                                ps[:, q * P:(q + 1) * P],
                                lhsT=hT[:, kk, tt * P:(tt + 1) * P],
                                rhs=wv_sb[:, kk, :],
                                start=(kk == 0), stop=(kk == KK - 1))
                    dst = vaug[:, b, tg * 4:(tg + 1) * 4, :].rearrange(
                        "p tt (h x) -> p tt h x", h=2)[:, :, :, 0:64]
                    srcv = ps[:].rearrange("p (tt h x) -> p tt h x",
                                           tt=4, h=2)
                    nc.vector.tensor_copy(dst, srcv)
                return u

            def qkv_units(b):
                us = []
                for j in range(4):
                    us.append(qk_unit(b, wq_sb, bq_sb, qT, j))
                    us.append(qk_unit(b, wk_sb, bk_sb, kT, j))
                for tg in range(4):
                    us.append(v_unit(b, tg))
                return us

            def rs_y(b):
                nc.gpsimd.collective_compute(
                    "ReduceScatter", ALU.add, replica_groups=RG,
                    ins=[rsy_in[b].opt()], outs=[rsy_out[b].opt()])

            # ---- software-pipelined attention over one batch ----
            # scores of chunk c overlap PVs of chunk c-1 (rolling lag D), so
            # the PE never waits on the exp round-trip; fillers absorb the
            # exp-rate deficit; normalize+Wo emitted at each chunk's last PV
            def normalize_and_wo(b, j, hd, att):
                jsl = slice(j * 512, (j + 1) * 512)
                h64 = slice(hd * 64, (hd + 1) * 64)
                den = small.tile([1, 512], BF16, tag="den")
                nc.vector.tensor_copy(den[:], att[64:65, :])
                rdp = wA.tile([P, 512], FP32, tag="wa", name="rdp")
                nc.tensor.matmul(rdp[0:64, :], lhsT=ones_b[:, 0:64],
                                 rhs=den[:], start=True, stop=True)
                rd = small.tile([64, 512], FP32, tag="rd")
                nc.vector.reciprocal_approx_fast(rd[:], rdp[0:64, :])
                nc.vector.tensor_mul(attnT[h64, b, jsl], att[0:64, :], rd[:])
                if hd == 1:
                    # inline Wo for this token chunk (local partial, all C)
                    for m in range(CM):
                        ps = wA.tile([P, 512], FP32, tag="wa", name="wops")
                        nc.tensor.matmul(ps[:], lhsT=wor_sb[:, m, :],
                                         rhs=attnT[:, b, jsl],
                                         start=True, stop=True)
                        st = stg.tile([P, 512], BF16, tag="wos")
                        nc.vector.tensor_copy(st[:], ps[:])
                        nc.sync.dma_start(rsy_in[b][m][:, jsl], st[:])

            def attention_batch(b, fillers, frate=5):
                NK = T // P
                pend = []   # (att, k, p, hd, j)

                def drain_burst(n):
                    # packed PV runs: same-PSUM-target matmuls back-to-back
                    # avoid the per-instruction bank-switch penalty
                    for _ in range(min(n, len(pend))):
                        att, k, p, hd, j = pend.pop(0)
                        nc.tensor.matmul(
                            att[:], lhsT=vaug[:, b, k, hd * 65:(hd + 1) * 65],
                            rhs=p[:], start=(k == 0), stop=(k == NK - 1))
                        if k == NK - 1:
                            normalize_and_wo(b, j, hd, att)

                it = 0
                for j in range(4):
                    for hd in range(2):
                        att = attq.tile([65, 512], FP32, tag="att",
                                        name=f"att{b}{j}{hd}")
                        h64 = slice(hd * 64, (hd + 1) * 64)
                        for k in range(NK):
                            it += 1
                            if fillers and it % frate == 0:
                                fillers.pop(0)()
                            sc = wA.tile([P, 512], FP32, tag="wa", name="sc")
                            nc.tensor.matmul(
                                sc[:], lhsT=kT[h64, b, k * P:(k + 1) * P],
                                rhs=qT[h64, b, j * 512:(j + 1) * 512],
                                start=True, stop=True)
                            p = psb.tile([P, 512], BF16, tag="p")
                            nc.scalar.activation(p[:], sc[:], AF.Exp,
                                                 scale=float(HS) ** -0.5)
                            pend.append((att, k, p, hd, j))
                            if len(pend) >= 12:
                                drain_burst(8)
                while pend:
                    drain_burst(99)

            # ---- FFN units (Megatron column/row parallel) ----
            uTt = {}

            def w1_unit(b, j, m):
                def u():
                    if m == 0:
                        uTt[(b, j)] = upool.tile([P, MT, 512], BF16, tag="u",
                                                 name=f"uT{b}{j}")
                    ps = wA.tile([P, 512], FP32, tag="wa", name="w1ps")
                    for kk in range(KK):
                        nc.tensor.matmul(
                            ps[:], lhsT=w1_sb[0][:, kk, m * P:(m + 1) * P],
                            rhs=h2full[b][:, kk, j * 512:(j + 1) * 512],
                            start=(kk == 0), stop=(kk == KK - 1))
                    nc.scalar.activation(uTt[(b, j)][:, m, :], ps[:], AF.Relu,
                                         bias=b1_sb[:, m:m + 1], scale=1.0)
                return u

            def w2_unit(b, j, mc):
                def u():
                    ps = wA.tile([P, 512], FP32, tag="wa", name="w2ps")
                    for q in range(MT):
                        nc.tensor.matmul(
                            ps[:], lhsT=w2_sb[0][:, q, mc * P:(mc + 1) * P],
                            rhs=uTt[(b, j)][:, q, :],
                            start=(q == 0), stop=(q == MT - 1))
                    st = stg.tile([P, 512], BF16, tag="zs")
                    zdrain()(st[:], ps[:])
                    nc.sync.dma_start(rsz_in[b][mc][:, j * 512:(j + 1) * 512],
                                      st[:])
                return u

            def ffn_units(b, j):
                return ([w1_unit(b, j, m) for m in range(MT)]
                        + [w2_unit(b, j, mc) for mc in range(CM)])

            def rs_z(b):
                nc.gpsimd.collective_compute(
                    "ReduceScatter", ALU.add, replica_groups=RG,
                    ins=[rsz_in[b].opt()], outs=[rsz_out[b].opt()])

            def final(b):
                zs = fin.tile([P, T], BF16, tag="zf", name=f"zf{b}")
                nc.sync.dma_start(zs[:], rsz_out[b][:])
                o = fin.tile([P, T], FP32, tag="o", name=f"o{b}")
                nc.vector.scalar_tensor_tensor(
                    out=o[:], in0=zs[:], scalar=b2_sb[:],
                    in1=yT[:, b, :], op0=ALU.add, op1=ALU.add)
                nc.sync.dma_start(out.ap()[:, b * T:(b + 1) * T], o[:])

            # =========== emission ===========
            with tc.tile_pool(name="xst", bufs=2) as xstp, \
                 tc.tile_pool(name="tpp", bufs=2, space="PSUM") as tpp, \
                 tc.tile_pool(name="hT", bufs=1) as hTp:

                # phase A: own-slice chunks first (critical path: stats AG)
                with tc.tile_pool(name="xc", bufs=1) as xcp:
                    xc_sb = {}
                    for b in range(B):
                        xc_sb[b] = xcp.tile([P, T // P, P], FP32, tag="xc",
                                            name=f"xc{b}")
                        nc.sync.dma_start(
                            xc_sb[b][:],
                            x_c.ap()[b * T:(b + 1) * T, :]
                            .rearrange("(tt p) c -> p tt c", p=P))
                    # full-x stages + weights (gpsimd queue, after xc)
                    xst = {}
                    for b in range(B):
                        for hf in range(2):
                            s = xstp.tile([P, 8, C], BF16, tag="xst",
                                          name=f"xst{b}{hf}")
                            nc.gpsimd.dma_start(
                                s[:],
                                x_bf.ap()[b * T + hf * 1024:
                                          b * T + (hf + 1) * 1024, :]
                                .rearrange("(tt p) c -> p tt c", p=P))
                            xst[(b, hf)] = s
                    wq_sb = ldconst(const, wq, [P, KK, P], BF16)
                    wk_sb = ldconst(const, wk, [P, KK, P], BF16)
                    wv_sb = ldconst(const, wv, [P, KK, P], BF16)
                    wor_sb = ldconst(const, wor, [P, CM, P], BF16)
                    bq_sb = ldconst(const, bqc, [P, 1])
                    bk_sb = ldconst(const, bkc, [P, 1])
                    bo_sb = ldconst(const, boc, [P, 1])
                    b1_sb = ldconst(const, b1c, [P, MT])
                    b2_sb = ldconst(const, b2c, [P, 1])
                    g2_sb = ldconst(const, g2, [P, 1])
                    be2_sb = ldconst(const, be2, [P, 1])
                    w1_sb = [ldconst(const, w1c, [P, KK, FL], BF16,
                                     eng=nc.scalar)]
                    w2_sb = [ldconst(const, w2c, [P, MT, C], BF16,
                                     eng=nc.scalar)]
                    nc.vector.memset(vaug[:, :, :, 64], 1.0)
                    nc.vector.memset(vaug[:, :, :, 129], 1.0)

                    for b in range(B):
                        for g in range(4):   # 4 transposes per wA tile
                            tp = wA.tile([P, 512], FP32, tag="wa",
                                         name=f"xtp{b}{g}")
                            for q in range(4):
                                tt = g * 4 + q
                                nc.tensor.transpose(
                                    tp[:, q * P:(q + 1) * P],
                                    xc_sb[b][:, tt, :], idf[:])
                            nc.vector.tensor_copy(
                                xT[:, b, g * 512:(g + 1) * 512], tp[:])
                        _ln_stats(nc, stats, xT[:, b, :], g1_sb[:], be1_sb[:],
                                  st_sb[:, 2 * b:2 * b + 1],
                                  st_sb[:, 2 * b + 1:2 * b + 2], T)
                    nc.sync.dma_start(st_in[:], st_sb[:])
                    nc.gpsimd.collective_compute(
                        "AllGather", ALU.bypass, replica_groups=RG,
                        ins=[st_in.opt()], outs=[st_out.opt()])
                    nc.sync.dma_start(
                        ag_sb[:], st_out.rearrange("(kk p) n -> p kk n", p=P))

                # transpose raw x^T into hT (no stats dep), apply in-place
                def trans_unit(b, kk, half):
                    def u():
                        hT = hT_of(b)
                        tp = tpp.tile([P, 1024], BF16, tag="tp")
                        for q in range(8):
                            tt = half * 8 + q
                            nc.tensor.transpose(
                                tp[:, q * P:(q + 1) * P],
                                xst[(b, tt // 8)][:, tt % 8,
                                                  kk * P:(kk + 1) * P],
                                idb[:])
                        nc.vector.tensor_copy(
                            hT[:, kk, half * 1024:(half + 1) * 1024], tp[:])
                    return u

                def apply_unit(b, kk):
                    def u():
                        hT = hT_of(b)
                        nc.vector.tensor_scalar(
                            out=hT[:, kk, :], in0=hT[:, kk, :],
                            scalar1=ag_sb[:, kk, 2 * b:2 * b + 1],
                            scalar2=ag_sb[:, kk, 2 * b + 1:2 * b + 2],
                            op0=ALU.mult, op1=ALU.add)
                    return u

                # b0: raw transposes, then stats-gated in-place applies
                for kk in range(KK):
                    for half in range(2):
                        trans_unit(0, kk, half)()
                for kk in range(KK):
                    apply_unit(0, kk)()
                for u in qkv_units(0):
                    u()

                # attention(b0): Wo inline per chunk; b1 prep as fillers
                fill_b0 = []
                for kk in range(KK):
                    for half in range(2):
                        fill_b0.append(trans_unit(1, kk, half))
                    fill_b0.append(apply_unit(1, kk))
                fill_b0.extend(qkv_units(1))
                attention_batch(0, fill_b0, frate=7)
                while fill_b0:
                    fill_b0.pop(0)()
                rs_y(0)

            with tc.tile_pool(name="h2f", bufs=1) as h2fp, \
                 tc.tile_pool(name="upool", bufs=2) as upool, \
                 tc.tile_pool(name="fin", bufs=2) as fin:
                # attention(b1): LN2(b0) as mid fillers; FFN(b0) runs
                # after attention, inside the RS-y(b1)/LN2/AG-h2(b1) gap
                l2u0 = ln2_units(0)
                fill_b1 = [(lambda: None)] * 3 + [l2u0[0], l2u0[1]]
                attention_batch(1, fill_b1, frate=13)
                while fill_b1:
                    fill_b1.pop(0)()
                rs_y(1)
                for u in ffn_units(0, 0):
                    u()
                l2u1 = ln2_units(1)
                for u in ffn_units(0, 1):
                    u()
                l2u1[0]()
                for u in ffn_units(0, 2):
                    u()
                l2u1[1]()
                for u in ffn_units(0, 3):
                    u()
                rs_z(0)
                for j in range(4):
                    for u in ffn_units(1, j):
                        u()
                rs_z(1)
                final(0)
                final(1)

    nc.compile()
    return nc


def prep_inputs(x, Wq, bq, Wk, bk, Wv, bv, Wo, bo, W1, b1, W2, b2,
                gamma1, beta1, gamma2, beta2):
    bf = ml_dtypes.bfloat16
    xf = np.asarray(x, np.float32).reshape(TN, C)
    x_bf_full = np.ascontiguousarray(xf.astype(bf))
    # softmax rows sum to 1, so the v bias is equivalent to adding
    # concat_h(bv) @ Wo to the attention-projection bias
    bo_eff = (np.asarray(bo, np.float64)
              + np.asarray(bv, np.float64).reshape(C) @ np.asarray(Wo, np.float64)
              ).astype(np.float32)
    Wo = np.asarray(Wo, np.float32)
    W1 = np.asarray(W1, np.float32)
    W2 = np.asarray(W2, np.float32)
    in_maps = []
    for i in range(NCORE):
        ci = slice(P * i, P * (i + 1))
        fi = slice(FL * i, FL * (i + 1))
        hA, hB = 2 * i, 2 * i + 1

        def tile_km(wcat):  # [C, 128] -> [p, kk, m]
            return np.ascontiguousarray(
                wcat.reshape(KK, P, P).transpose(1, 0, 2)).astype(bf)

        wq_cat = np.concatenate([Wq[hA], Wq[hB]], axis=1)
        wk_cat = np.concatenate([Wk[hA], Wk[hB]], axis=1)
        wv_cat = np.concatenate([Wv[hA], Wv[hB]], axis=1)
        in_maps.append({
            "x_bf": x_bf_full,
            "x_c": np.ascontiguousarray(xf[:, ci]),
            "wq": tile_km(wq_cat),
            "wk": tile_km(wk_cat),
            "wv": tile_km(wv_cat),
            "wor": np.ascontiguousarray(
                Wo[ci, :].reshape(P, CM, P)).astype(bf),
            "w1c": np.ascontiguousarray(
                W1[:, fi].reshape(KK, P, FL).transpose(1, 0, 2)).astype(bf),
            "w2c": np.ascontiguousarray(
                W2[fi, :].reshape(MT, P, C).transpose(1, 0, 2)).astype(bf),
            "bqc": np.concatenate([bq[hA], bq[hB]])[:, None].astype(np.float32),
            "bkc": np.concatenate([bk[hA], bk[hB]])[:, None].astype(np.float32),
            "boc": bo_eff[ci][:, None].astype(np.float32),
            "b1c": np.ascontiguousarray(
                np.asarray(b1)[fi].reshape(MT, P).T).astype(np.float32),
            "b2c": np.asarray(b2)[ci][:, None].astype(np.float32),
            "g1": np.asarray(gamma1)[ci][:, None].astype(np.float32),
            "be1": np.asarray(beta1)[ci][:, None].astype(np.float32),
            "g2": np.asarray(gamma2)[ci][:, None].astype(np.float32),
            "be2": np.asarray(beta2)[ci][:, None].astype(np.float32),
        })
    return in_maps


def kernel(**inputs):
    inputs = {k: np.asarray(v) for k, v in inputs.items()}
    if "nc" not in _cache:
        _cache["nc"] = build()
    nc = _cache["nc"]
    in_maps = prep_inputs(**inputs)
    res = bass_utils.run_bass_kernel_spmd(nc, in_maps, core_ids=list(range(NCORE)))
    full = np.concatenate([res.results[i]["out"] for i in range(NCORE)], axis=0)
    return np.ascontiguousarray(full.T).reshape(B, T, C).astype(np.float32)
